# revision 22
# baseline (speedup 1.0000x reference)
# Trainium2 Bass kernel for nn_Apply_on_single_area (vmapped per-area loss).
#
# Layout: one area per SBUF partition (128 areas per "chunk"), G=8 chunks per
# "group" so batched elementwise ops amortize instruction overhead. Per-area
# reductions use fused tensor_tensor_reduce (product+sum in one DVE op).
# Engines: DVE (products/reduces), ACT (squares/relu/copies/final affines),
# GPSIMD (per-area scalar broadcast multiply for out_image), HWDGE DMA.
#
# Self-contained: does not read /root/problem/*; shapes are hardcoded.
import functools
import os
import sys

import numpy as np

for _p in ("/opt/trn_rl_repo", "/root/.axon_site/_ro/trn_rl_repo"):
    if os.path.isdir(_p) and _p not in sys.path:
        sys.path.append(_p)

import concourse.bass as bass  # noqa: E402
import concourse.bacc as bacc  # noqa: E402
import concourse.tile as tile  # noqa: E402
from concourse import mybir  # noqa: E402
from concourse.bass_utils import run_bass_kernel_spmd  # noqa: E402

P = 128  # SBUF partitions = areas per chunk
NCORES = 8
DT = mybir.dt.float32
F = mybir.ActivationFunctionType
ALU = mybir.AluOpType
AX = mybir.AxisListType

SQ_I = float(np.sqrt(1.25))  # consistency: 5*(0.5*dmo)^2 = 1.25*dmo^2 (all rows)


def make_diamond() -> np.ndarray:
    """numpy replica of reference.get_diamond(15) padded to (16,16)."""
    d = 15
    r = d // 2
    a = np.zeros((d, 2 * d), np.float32)
    for idx in range(d):
        o = abs(r - idx)
        a[idx, o : o + d] = 1.0
    b = np.flip(a)[:, d:]
    a = a[:, :-d]
    dm = np.logical_and(a.astype(bool), b.astype(bool)).astype(np.float32)
    out = np.zeros((16, 16), np.float32)
    out[:15, :15] = dm
    return out


DIAM = make_diamond()
K_D2 = float(np.sum(DIAM * DIAM))  # sum d^2 (d is 0/1 so == sum d)


def build_nc(npc: int, g: int):
    """Build the Bass module for one core processing npc areas, g chunks/group."""
    assert npc % (P * g) == 0
    ng = npc // (P * g)
    nch = npc // P  # total chunks

    nc = bacc.Bacc("TRN2", debug=False, target_bir_lowering=False)
    x_im = nc.dram_tensor("x_im", [npc, 256], DT, kind="ExternalInput").ap()
    x_mn = nc.dram_tensor("x_mn", [npc, 128], DT, kind="ExternalInput").ap()
    x_mo = nc.dram_tensor("x_mo", [npc, 128], DT, kind="ExternalInput").ap()
    c_dm = nc.dram_tensor("c_dm", [P, 256], DT, kind="ExternalInput").ap()
    o_mc = nc.dram_tensor("o_mc", [npc, 256], DT, kind="ExternalOutput").ap()
    o_im = nc.dram_tensor("o_im", [npc, 256], DT, kind="ExternalOutput").ap()
    o_sc = nc.dram_tensor("o_sc", [npc, 8], DT, kind="ExternalOutput").ap()

    im_r = x_im.rearrange("(n p) x -> p n x", p=P)
    mn_r = x_mn.rearrange("(n p) x -> p n x", p=P)
    mo_r = x_mo.rearrange("(n p) x -> p n x", p=P)
    omc_r = o_mc.rearrange("(n p) x -> p n x", p=P)
    oim_r = o_im.rearrange("(n p) x -> p n x", p=P)
    osc_r = o_sc.rearrange("(n p) x -> p n x", p=P)

    with tile.TileContext(nc) as tc:
        from contextlib import ExitStack

        with ExitStack() as ctx:
            pio = ctx.enter_context(tc.tile_pool(name="pio", bufs=2))
            pm2 = ctx.enter_context(tc.tile_pool(name="pm2", bufs=2))
            pmid = ctx.enter_context(tc.tile_pool(name="pmid", bufs=1))
            pscr = ctx.enter_context(tc.tile_pool(name="pscr", bufs=2))
            pglob = ctx.enter_context(tc.tile_pool(name="pglob", bufs=1))
            ptiny = ctx.enter_context(tc.tile_pool(name="ptiny", bufs=2))

            diam = pglob.tile([P, 256], DT, tag="diam")
            nc.sync.dma_start(out=diam, in_=c_dm)

            # scalar staging, one column set per chunk:
            # 0 cons, 1 round, 2 fvar, 3 edge, 4 avgcov
            stag = pglob.tile([P, nch, 8], DT, tag="stag")
            nc.scalar.memzero(stag)
            # per-chunk accumulated values (filled by TTR/STT/act accум outs)
            # glob1: 0-3 P_t, 4-7 A_t, 8-11 B_t, 12 p0(sum mc*im), 13 pd(sum mc*d),
            #        14 e2(edge raw), 15 cons(raw relu sum)
            glob1 = pglob.tile([P, nch, 16], DT, tag="glob1")
            # glob2: 0 r01 1 r1415 2 c01 3 c1415 (of mc); 4-7 same of m2; 8 S0; 9 M0
            glob2 = pglob.tile([P, nch, 12], DT, tag="glob2")

            for grp in range(ng):
                g0 = grp * g
                im_g = pio.tile([P, g, 256], DT, tag="im")
                mn_g = pio.tile([P, g, 128], DT, tag="mn")
                mo_g = pio.tile([P, g, 128], DT, tag="mo")
                nc.sync.dma_start(out=im_g, in_=im_r[:, g0 : g0 + g, :])
                nc.sync.dma_start(out=mn_g, in_=mn_r[:, g0 : g0 + g, :])
                nc.sync.dma_start(out=mo_g, in_=mo_r[:, g0 : g0 + g, :])

                mc_g = pio.tile([P, g, 256], DT, tag="mc")
                img_g = pio.tile([P, g, 256], DT, tag="img")

                # interleave: mc[2i]=mn[i], mc[2i+1]=mo[i] (rows of 16)
                mc4 = mc_g.rearrange("p g (r t c) -> p g r t c", t=2, c=16)
                nc.scalar.copy(
                    out=mc4[:, :, :, 0, :],
                    in_=mn_g.rearrange("p g (r c) -> p g r c", c=16),
                )
                nc.scalar.copy(
                    out=mc4[:, :, :, 1, :],
                    in_=mo_g.rearrange("p g (r c) -> p g r c", c=16),
                )

                m2_g = pm2.tile([P, g, 256], DT, tag="m2")
                m4_g = pmid.tile([P, g, 256], DT, tag="m4")
                i2_g = pmid.tile([P, g, 256], DT, tag="i2")
                m3_g = pmid.tile([P, g, 256], DT, tag="m3")
                nc.scalar.square(out=m2_g, in_=mc_g)
                nc.scalar.square(out=m4_g, in_=m2_g)
                nc.scalar.square(out=i2_g, in_=im_g)
                nc.vector.tensor_mul(m3_g, mc_g, m2_g)

                # ---- batched partial sums into glob2 ----
                gl2 = glob2[:, g0 : g0 + g, :]
                mc_rc = mc_g.rearrange("p g (r c) -> p g r c", c=16)
                m2_rc = m2_g.rearrange("p g (r c) -> p g r c", c=16)
                nc.vector.reduce_sum(out=gl2[:, :, 8], in_=mc_g, axis=AX.X)
                nc.vector.reduce_sum(out=gl2[:, :, 9], in_=m2_g, axis=AX.X)
                nc.vector.reduce_sum(out=gl2[:, :, 0], in_=mc_g[:, :, 0:32], axis=AX.X)
                nc.vector.reduce_sum(
                    out=gl2[:, :, 1], in_=mc_g[:, :, 224:256], axis=AX.X
                )
                nc.vector.reduce_sum(
                    out=gl2[:, :, 2], in_=mc_rc[:, :, :, 0:2], axis=AX.XY
                )
                nc.vector.reduce_sum(
                    out=gl2[:, :, 3], in_=mc_rc[:, :, :, 14:16], axis=AX.XY
                )
                nc.vector.reduce_sum(out=gl2[:, :, 4], in_=m2_g[:, :, 0:32], axis=AX.X)
                nc.vector.reduce_sum(
                    out=gl2[:, :, 5], in_=m2_g[:, :, 224:256], axis=AX.X
                )
                nc.vector.reduce_sum(
                    out=gl2[:, :, 6], in_=m2_rc[:, :, :, 0:2], axis=AX.XY
                )
                nc.vector.reduce_sum(
                    out=gl2[:, :, 7], in_=m2_rc[:, :, :, 14:16], axis=AX.XY
                )

                # ---- consistency: z = mn - (1.25*g^2 + 5*mo), then relu-sum ----
                q_g = pmid.tile([P, g, 128], DT, tag="q")
                gin = pscr.tile([P, g, 96], DT, tag="gin")
                mo_rc = mo_g.rearrange("p g (r c) -> p g r c", c=16)
                q_rc = q_g.rearrange("p g (r c) -> p g r c", c=16)
                nc.vector.tensor_sub(gin, mo_g[:, :, 32:128], mo_g[:, :, 0:96])
                # q = 1.25*d^2 via ScalarE square with baked scale
                nc.scalar.activation(
                    out=q_g[:, :, 16:112], in_=gin, func=F.Square, scale=SQ_I
                )
                nc.scalar.activation(
                    out=q_rc[:, :, 0:8:7, :],
                    in_=mo_rc[:, :, 1:7:5, :],
                    func=F.Square,
                    scale=SQ_I,
                )
                # t = q + 5*mo ; z = mn - t
                mo5 = pscr.tile([P, g, 128], DT, tag="mo5")
                nc.scalar.activation(out=mo5, in_=mo_g, func=F.Copy, scale=5.0)
                t_g = pmid.tile([P, g, 128], DT, tag="tcons")
                nc.vector.tensor_add(t_g, q_g, mo5)
                z_g = pmid.tile([P, g, 128], DT, tag="zcons")
                nc.vector.tensor_sub(z_g, mn_g, t_g)

                # ---- edge buffers: dif = im - mc, raw row-grads Di (im), D (dif) ----
                df_g = pmid.tile([P, g, 256], DT, tag="df")
                di_g = pmid.tile([P, g, 256], DT, tag="di")
                dd_g = pmid.tile([P, g, 256], DT, tag="dd")
                nc.vector.tensor_sub(df_g, im_g, mc_g)
                im_rc = im_g.rearrange("p g (r c) -> p g r c", c=16)
                di_rc = di_g.rearrange("p g (r c) -> p g r c", c=16)
                df_rc = df_g.rearrange("p g (r c) -> p g r c", c=16)
                dd_rc = dd_g.rearrange("p g (r c) -> p g r c", c=16)
                nc.vector.tensor_sub(
                    di_g[:, :, 16:240], im_g[:, :, 32:256], im_g[:, :, 0:224]
                )
                nc.vector.tensor_sub(
                    di_rc[:, :, 0:16:15, :],
                    im_rc[:, :, 1:16:14, :],
                    im_rc[:, :, 0:15:14, :],
                )
                nc.vector.tensor_sub(
                    dd_g[:, :, 16:240], df_g[:, :, 32:256], df_g[:, :, 0:224]
                )
                nc.vector.tensor_sub(
                    dd_rc[:, :, 0:16:15, :],
                    df_rc[:, :, 1:16:14, :],
                    df_rc[:, :, 0:15:14, :],
                )
                # D^2 in place with edge-loss weights baked in:
                # interior 0.0625*D^2, edge rows 0.5*D^2
                nc.scalar.activation(
                    out=dd_g[:, :, 16:240],
                    in_=dd_g[:, :, 16:240],
                    func=F.Square,
                    scale=0.25,
                )
                nc.scalar.activation(
                    out=dd_rc[:, :, 0:16:15, :],
                    in_=dd_rc[:, :, 0:16:15, :],
                    func=F.Square,
                    scale=float(np.sqrt(0.5)),
                )

                # ---- per-chunk fused product+sum (STT with accum) ----
                gl1 = glob1[:, g0 : g0 + g, :]
                for cg in range(g):
                    scr = pscr.tile([P, 256], DT, tag="scrv")
                    scr_rc = scr.rearrange("p (r c) -> p r c", c=16)
                    imC = im_g[:, cg]
                    mcC = mc_g[:, cg]
                    i2C = i2_g[:, cg]
                    m4C = m4_g[:, cg]
                    m3C = m3_g[:, cg]
                    imR = imC.rearrange("p (r c) -> p r c", c=16)
                    mcR = mcC.rearrange("p (r c) -> p r c", c=16)
                    i2R = i2C.rearrange("p (r c) -> p r c", c=16)
                    m4R = m4C.rearrange("p (r c) -> p r c", c=16)
                    m3R = m3C.rearrange("p (r c) -> p r c", c=16)

                    def psum(out_ap, a, b, acc):
                        nc.vector.scalar_tensor_tensor(
                            out=out_ap,
                            in0=a,
                            scalar=1.0,
                            in1=b,
                            op0=ALU.mult,
                            op1=ALU.mult,
                            accum_out=acc,
                        )

                    # P_t = sum im_sh * mc_sh  (up, down, left, right)
                    psum(scr[:, 0:224], imC[:, 0:224], mcC[:, 32:256], gl1[:, cg, 0:1])
                    psum(scr[:, 0:224], imC[:, 32:256], mcC[:, 0:224], gl1[:, cg, 1:2])
                    psum(
                        scr_rc[:, :, 0:14],
                        imR[:, :, 0:14],
                        mcR[:, :, 2:16],
                        gl1[:, cg, 2:3],
                    )
                    psum(
                        scr_rc[:, :, 0:14],
                        imR[:, :, 2:16],
                        mcR[:, :, 0:14],
                        gl1[:, cg, 3:4],
                    )
                    # A_t = sum im^2 * m^4
                    psum(scr[:, 0:224], i2C[:, 0:224], m4C[:, 32:256], gl1[:, cg, 4:5])
                    psum(scr[:, 0:224], i2C[:, 32:256], m4C[:, 0:224], gl1[:, cg, 5:6])
                    psum(
                        scr_rc[:, :, 0:14],
                        i2R[:, :, 0:14],
                        m4R[:, :, 2:16],
                        gl1[:, cg, 6:7],
                    )
                    psum(
                        scr_rc[:, :, 0:14],
                        i2R[:, :, 2:16],
                        m4R[:, :, 0:14],
                        gl1[:, cg, 7:8],
                    )
                    # B_t = sum im * m^3
                    psum(scr[:, 0:224], imC[:, 0:224], m3C[:, 32:256], gl1[:, cg, 8:9])
                    psum(scr[:, 0:224], imC[:, 32:256], m3C[:, 0:224], gl1[:, cg, 9:10])
                    psum(
                        scr_rc[:, :, 0:14],
                        imR[:, :, 0:14],
                        m3R[:, :, 2:16],
                        gl1[:, cg, 10:11],
                    )
                    psum(
                        scr_rc[:, :, 0:14],
                        imR[:, :, 2:16],
                        m3R[:, :, 0:14],
                        gl1[:, cg, 11:12],
                    )
                    # p0 = sum mc*im ; pd = sum mc*diamond
                    psum(scr, mcC, imC, gl1[:, cg, 12:13])
                    psum(scr, mcC, diam, gl1[:, cg, 13:14])
                    # edge: weights already baked into dd (= scaled D^2)
                    psum(scr, dd_g[:, cg], di_g[:, cg], gl1[:, cg, 14:15])
                    # consistency relu + accumulate (ScalarE)
                    scr_s = pscr.tile([P, 128], DT, tag="scrs")
                    nc.scalar.activation(
                        out=scr_s,
                        in_=z_g[:, cg],
                        func=F.Relu,
                        accum_out=gl1[:, cg, 15:16],
                    )

                # ---- out_image: ratio = p0 / S0 per chunk, img = mc * ratio ----
                rs0 = ptiny.tile([P, g], DT, tag="rs0")
                ratio = ptiny.tile([P, g], DT, tag="ratio")
                nc.vector.reciprocal(out=rs0, in_=gl2[:, :, 8])
                nc.vector.tensor_mul(ratio, gl1[:, :, 12], rs0)
                for cg in range(g):
                    nc.scalar.activation(
                        out=img_g[:, cg],
                        in_=mc_g[:, cg],
                        func=F.Copy,
                        scale=ratio[:, cg : cg + 1],
                    )

                nc.sync.dma_start(out=omc_r[:, g0 : g0 + g, :], in_=mc_g)
                nc.sync.dma_start(out=oim_r[:, g0 : g0 + g, :], in_=img_g)

            # ================= final scalar math over all chunks =================
            drv = pglob.tile([P, nch, 16], DT, tag="drv")
            # s_t = S0 - partial ; C_t = M0 - partial(m2)   (t: up,down,left,right)
            for t in range(4):
                nc.vector.tensor_sub(
                    drv[:, :, t], glob2[:, :, 8], glob2[:, :, t]
                )  # s_t
                nc.vector.tensor_sub(
                    drv[:, :, 4 + t], glob2[:, :, 9], glob2[:, :, 4 + t]
                )  # C_t
            # rs_t = 1/s_t
            nc.vector.reciprocal(out=drv[:, :, 8:12], in_=drv[:, :, 0:4])
            # mu_t = P_t * rs_t
            nc.vector.tensor_mul(drv[:, :, 12:16], glob1[:, :, 0:4], drv[:, :, 8:12])

            drv2 = pglob.tile([P, nch, 16], DT, tag="drv2")
            mu = drv[:, :, 12:16]
            # t1 = mu*B ; t2 = A - 2*t1 ; musq = mu*mu ; t4 = musq*C ; t5 = t2+t4
            nc.vector.tensor_mul(drv2[:, :, 0:4], mu, glob1[:, :, 8:12])
            nc.vector.tensor_add(drv2[:, :, 0:4], drv2[:, :, 0:4], drv2[:, :, 0:4])
            nc.vector.tensor_sub(drv2[:, :, 4:8], glob1[:, :, 4:8], drv2[:, :, 0:4])
            nc.vector.tensor_mul(drv2[:, :, 8:12], mu, mu)  # musq
            nc.vector.tensor_mul(drv2[:, :, 12:16], drv2[:, :, 8:12], drv[:, :, 4:8])
            nc.vector.tensor_add(drv2[:, :, 4:8], drv2[:, :, 4:8], drv2[:, :, 12:16])
            # var_t = t5 * rs_t
            nc.vector.tensor_mul(drv2[:, :, 0:4], drv2[:, :, 4:8], drv[:, :, 8:12])

            fin = pglob.tile([P, nch, 8], DT, tag="fin")
            # sums over t of mu, mu^2, v, v^2
            nc.vector.reduce_sum(out=fin[:, :, 0], in_=mu, axis=AX.X)
            nc.vector.reduce_sum(out=fin[:, :, 1], in_=drv2[:, :, 8:12], axis=AX.X)
            nc.vector.reduce_sum(out=fin[:, :, 2], in_=drv2[:, :, 0:4], axis=AX.X)
            nc.vector.tensor_mul(drv2[:, :, 12:16], drv2[:, :, 0:4], drv2[:, :, 0:4])
            nc.vector.reduce_sum(out=fin[:, :, 3], in_=drv2[:, :, 12:16], axis=AX.X)
            # fvar = (smu2+sv2)/8 - (smu^2 + sv^2)/32
            nc.vector.tensor_add(fin[:, :, 4], fin[:, :, 1], fin[:, :, 3])
            nc.vector.tensor_mul(fin[:, :, 5], fin[:, :, 0], fin[:, :, 0])
            nc.vector.tensor_mul(fin[:, :, 6], fin[:, :, 2], fin[:, :, 2])
            # fvar = (smu2+sv2)/8 - (smu^2+sv^2)/32 = (4*tA - tD)/32
            nc.vector.tensor_add(fin[:, :, 5], fin[:, :, 5], fin[:, :, 6])  # tD
            nc.vector.tensor_add(fin[:, :, 4], fin[:, :, 4], fin[:, :, 4])  # 2*tA
            nc.vector.tensor_add(fin[:, :, 4], fin[:, :, 4], fin[:, :, 4])  # 4*tA
            nc.vector.tensor_sub(fin[:, :, 4], fin[:, :, 4], fin[:, :, 5])
            nc.scalar.activation(
                out=stag[:, :, 2], in_=fin[:, :, 4], func=F.Copy, scale=1.0 / 32.0
            )
            # consistency = relu_sum / 128
            nc.scalar.activation(
                out=stag[:, :, 0], in_=glob1[:, :, 15], func=F.Copy, scale=1.0 / 128.0
            )
            # rounding = (S0 - M0)/64 - 1
            nc.vector.tensor_sub(fin[:, :, 7], glob2[:, :, 8], glob2[:, :, 9])
            nc.scalar.activation(
                out=stag[:, :, 1],
                in_=fin[:, :, 7],
                func=F.Copy,
                scale=1.0 / 64.0,
                bias=-1.0,
            )
            # edge = e2 / 256
            nc.scalar.activation(
                out=stag[:, :, 3], in_=glob1[:, :, 14], func=F.Copy, scale=1.0 / 256.0
            )
            # avg_cov = (M0 - 2*pd + K) * 0.5/256
            nc.vector.tensor_add(fin[:, :, 6], glob1[:, :, 13], glob1[:, :, 13])
            nc.vector.tensor_sub(fin[:, :, 6], glob2[:, :, 9], fin[:, :, 6])
            nc.scalar.activation(
                out=stag[:, :, 4],
                in_=fin[:, :, 6],
                func=F.Copy,
                scale=0.5 / 256.0,
                bias=K_D2 * 0.5 / 256.0,
            )

            nc.sync.dma_start(out=osc_r, in_=stag)

    nc.compile()
    return nc


@functools.lru_cache(maxsize=2)
def _get_nc(npc: int, g: int):
    return build_nc(npc, g)


def _pick_g(npc: int) -> int:
    for g in (8, 4, 2, 1):
        if npc % (P * g) == 0:
            return g
    raise ValueError(f"npc={npc} not divisible by {P}")


def run_cores(resized_image, mask_new, mask_old, trace=False, ncores=NCORES):
    B = resized_image.shape[0]
    assert B % ncores == 0
    npc = B // ncores
    nc = _get_nc(npc, _pick_g(npc))
    im = np.ascontiguousarray(resized_image.reshape(B, 256), dtype=np.float32)
    mn = np.ascontiguousarray(mask_new.reshape(B, 128), dtype=np.float32)
    mo = np.ascontiguousarray(mask_old.reshape(B, 128), dtype=np.float32)
    dm = np.ascontiguousarray(np.tile(DIAM.reshape(1, 256), (P, 1)))
    in_maps = [
        {
            "x_im": im[c * npc : (c + 1) * npc],
            "x_mn": mn[c * npc : (c + 1) * npc],
            "x_mo": mo[c * npc : (c + 1) * npc],
            "c_dm": dm,
        }
        for c in range(ncores)
    ]
    res = run_bass_kernel_spmd(
        nc, in_maps, core_ids=list(range(ncores)), trace=trace
    )
    outs = res.results
    mc = np.concatenate([r["o_mc"] for r in outs]).reshape(B, 16, 16)
    oi = np.concatenate([r["o_im"] for r in outs]).reshape(B, 16, 16)
    sc = np.concatenate([r["o_sc"] for r in outs])
    result = (
        mc,
        oi,
        np.ascontiguousarray(sc[:, 0]),
        np.ascontiguousarray(sc[:, 1]),
        np.ascontiguousarray(sc[:, 2]),
        np.ascontiguousarray(sc[:, 3]),
        np.ascontiguousarray(sc[:, 4]),
    )
    return result, res


def kernel(resized_image, mask_new, mask_old, **_unused):
    result, _ = run_cores(resized_image, mask_new, mask_old)
    return result


# revision 27
# speedup vs baseline: 1.0212x; 1.0212x over previous
# Trainium2 Bass kernel for nn_Apply_on_single_area (vmapped per-area loss).
#
# Layout: one area per SBUF partition (128 areas per "chunk"), G=8 chunks per
# "group" so batched elementwise ops amortize instruction overhead. Per-area
# reductions use fused tensor_tensor_reduce (product+sum in one DVE op).
# Engines: DVE (products/reduces), ACT (squares/relu/copies/final affines),
# GPSIMD (per-area scalar broadcast multiply for out_image), HWDGE DMA.
#
# Self-contained: does not read /root/problem/*; shapes are hardcoded.
import functools
import os
import sys

import numpy as np

for _p in ("/opt/trn_rl_repo", "/root/.axon_site/_ro/trn_rl_repo"):
    if os.path.isdir(_p) and _p not in sys.path:
        sys.path.append(_p)

import concourse.bass as bass  # noqa: E402
import concourse.bacc as bacc  # noqa: E402
import concourse.tile as tile  # noqa: E402
from concourse import mybir  # noqa: E402
from concourse.bass_utils import run_bass_kernel_spmd  # noqa: E402

P = 128  # SBUF partitions = areas per chunk
NCORES = 8
DT = mybir.dt.float32
F = mybir.ActivationFunctionType
ALU = mybir.AluOpType
AX = mybir.AxisListType

SQ_I = float(np.sqrt(1.25))  # consistency: 5*(0.5*dmo)^2 = 1.25*dmo^2 (all rows)


def make_diamond() -> np.ndarray:
    """numpy replica of reference.get_diamond(15) padded to (16,16)."""
    d = 15
    r = d // 2
    a = np.zeros((d, 2 * d), np.float32)
    for idx in range(d):
        o = abs(r - idx)
        a[idx, o : o + d] = 1.0
    b = np.flip(a)[:, d:]
    a = a[:, :-d]
    dm = np.logical_and(a.astype(bool), b.astype(bool)).astype(np.float32)
    out = np.zeros((16, 16), np.float32)
    out[:15, :15] = dm
    return out


DIAM = make_diamond()
K_D2 = float(np.sum(DIAM * DIAM))  # sum d^2 (d is 0/1 so == sum d)


def build_nc(npc: int, g: int):
    """Build the Bass module for one core processing npc areas, g chunks/group."""
    assert npc % (P * g) == 0
    ng = npc // (P * g)
    nch = npc // P  # total chunks

    nc = bacc.Bacc("TRN2", debug=False, target_bir_lowering=False)
    x_im = nc.dram_tensor("x_im", [npc, 256], DT, kind="ExternalInput").ap()
    x_mn = nc.dram_tensor("x_mn", [npc, 128], DT, kind="ExternalInput").ap()
    x_mo = nc.dram_tensor("x_mo", [npc, 128], DT, kind="ExternalInput").ap()
    c_dm = nc.dram_tensor("c_dm", [P, 256], DT, kind="ExternalInput").ap()
    o_mc = nc.dram_tensor("o_mc", [npc, 256], DT, kind="ExternalOutput").ap()
    o_im = nc.dram_tensor("o_im", [npc, 256], DT, kind="ExternalOutput").ap()
    o_sc = nc.dram_tensor("o_sc", [npc, 8], DT, kind="ExternalOutput").ap()

    im_r = x_im.rearrange("(n p) x -> p n x", p=P)
    mn_r = x_mn.rearrange("(n p) x -> p n x", p=P)
    mo_r = x_mo.rearrange("(n p) x -> p n x", p=P)
    omc_r = o_mc.rearrange("(n p) x -> p n x", p=P)
    oim_r = o_im.rearrange("(n p) x -> p n x", p=P)
    osc_r = o_sc.rearrange("(n p) x -> p n x", p=P)

    with tile.TileContext(nc) as tc:
        from contextlib import ExitStack

        with ExitStack() as ctx:
            pio = ctx.enter_context(tc.tile_pool(name="pio", bufs=2))
            pm2 = ctx.enter_context(tc.tile_pool(name="pm2", bufs=2))
            pmid = ctx.enter_context(tc.tile_pool(name="pmid", bufs=1))
            pscr = ctx.enter_context(tc.tile_pool(name="pscr", bufs=1))
            pglob = ctx.enter_context(tc.tile_pool(name="pglob", bufs=1))
            ptiny = ctx.enter_context(tc.tile_pool(name="ptiny", bufs=2))

            diam = pglob.tile([P, 256], DT, tag="diam")
            nc.sync.dma_start(out=diam, in_=c_dm)

            # scalar staging, one column set per chunk:
            # 0 cons, 1 round, 2 fvar, 3 edge, 4 avgcov
            stag = pglob.tile([P, nch, 8], DT, tag="stag")
            nc.scalar.memzero(stag)
            # per-chunk accumulated values (filled by TTR/STT/act accум outs)
            # glob1: 0-3 P_t, 4-7 A_t, 8-11 B_t, 12 p0(sum mc*im), 13 pd(sum mc*d),
            #        14 e2(edge raw), 15 cons(raw relu sum)
            glob1 = pglob.tile([P, nch, 16], DT, tag="glob1")
            # glob2: 0 r01 1 r1415 2 c01 3 c1415 (of mc); 4-7 same of m2; 8 S0; 9 M0
            glob2 = pglob.tile([P, nch, 12], DT, tag="glob2")

            for grp in range(ng):
                g0 = grp * g
                im_g = pio.tile([P, g, 256], DT, tag="im")
                mn_g = pio.tile([P, g, 128], DT, tag="mn")
                mo_g = pio.tile([P, g, 128], DT, tag="mo")
                nc.sync.dma_start(out=im_g, in_=im_r[:, g0 : g0 + g, :])
                nc.sync.dma_start(out=mn_g, in_=mn_r[:, g0 : g0 + g, :])
                nc.sync.dma_start(out=mo_g, in_=mo_r[:, g0 : g0 + g, :])

                mc_g = pio.tile([P, g, 256], DT, tag="mc")
                img_g = pio.tile([P, g, 256], DT, tag="img")

                # interleave: mc[2i]=mn[i], mc[2i+1]=mo[i] (rows of 16)
                mc4 = mc_g.rearrange("p g (r t c) -> p g r t c", t=2, c=16)
                nc.scalar.copy(
                    out=mc4[:, :, :, 0, :],
                    in_=mn_g.rearrange("p g (r c) -> p g r c", c=16),
                )
                nc.scalar.copy(
                    out=mc4[:, :, :, 1, :],
                    in_=mo_g.rearrange("p g (r c) -> p g r c", c=16),
                )

                m2_g = pm2.tile([P, g, 256], DT, tag="m2")
                m4_g = pmid.tile([P, g, 256], DT, tag="m4")
                i2_g = pmid.tile([P, g, 256], DT, tag="i2")
                m3_g = pmid.tile([P, g, 256], DT, tag="m3")
                nc.scalar.square(out=m2_g, in_=mc_g)
                nc.scalar.square(out=m4_g, in_=m2_g)
                nc.scalar.square(out=i2_g, in_=im_g)
                nc.vector.tensor_mul(m3_g, mc_g, m2_g)

                # ---- batched partial sums into glob2 ----
                gl2 = glob2[:, g0 : g0 + g, :]
                mc_rc = mc_g.rearrange("p g (r c) -> p g r c", c=16)
                m2_rc = m2_g.rearrange("p g (r c) -> p g r c", c=16)
                nc.vector.reduce_sum(out=gl2[:, :, 8], in_=mc_g, axis=AX.X)
                nc.vector.reduce_sum(out=gl2[:, :, 9], in_=m2_g, axis=AX.X)
                nc.vector.reduce_sum(out=gl2[:, :, 0], in_=mc_g[:, :, 0:32], axis=AX.X)
                nc.vector.reduce_sum(
                    out=gl2[:, :, 1], in_=mc_g[:, :, 224:256], axis=AX.X
                )
                nc.vector.reduce_sum(
                    out=gl2[:, :, 2], in_=mc_rc[:, :, :, 0:2], axis=AX.XY
                )
                nc.vector.reduce_sum(
                    out=gl2[:, :, 3], in_=mc_rc[:, :, :, 14:16], axis=AX.XY
                )
                nc.vector.reduce_sum(out=gl2[:, :, 4], in_=m2_g[:, :, 0:32], axis=AX.X)
                nc.vector.reduce_sum(
                    out=gl2[:, :, 5], in_=m2_g[:, :, 224:256], axis=AX.X
                )
                nc.vector.reduce_sum(
                    out=gl2[:, :, 6], in_=m2_rc[:, :, :, 0:2], axis=AX.XY
                )
                nc.vector.reduce_sum(
                    out=gl2[:, :, 7], in_=m2_rc[:, :, :, 14:16], axis=AX.XY
                )

                # ---- consistency: z = mn - (1.25*g^2 + 5*mo), then relu-sum ----
                q_g = pmid.tile([P, g, 128], DT, tag="q")
                gin = pscr.tile([P, g, 96], DT, tag="gin")
                mo_rc = mo_g.rearrange("p g (r c) -> p g r c", c=16)
                q_rc = q_g.rearrange("p g (r c) -> p g r c", c=16)
                nc.vector.tensor_sub(gin, mo_g[:, :, 32:128], mo_g[:, :, 0:96])
                # q = 1.25*d^2 via ScalarE square with baked scale
                nc.scalar.activation(
                    out=q_g[:, :, 16:112], in_=gin, func=F.Square, scale=SQ_I
                )
                nc.scalar.activation(
                    out=q_rc[:, :, 0:8:7, :],
                    in_=mo_rc[:, :, 1:7:5, :],
                    func=F.Square,
                    scale=SQ_I,
                )
                # t = q + 5*mo ; z = mn - t
                mo5 = pscr.tile([P, g, 128], DT, tag="mo5")
                nc.scalar.activation(out=mo5, in_=mo_g, func=F.Copy, scale=5.0)
                t_g = pmid.tile([P, g, 128], DT, tag="tcons")
                nc.vector.tensor_add(t_g, q_g, mo5)
                z_g = pmid.tile([P, g, 128], DT, tag="zcons")
                nc.vector.tensor_sub(z_g, mn_g, t_g)

                # ---- edge buffers: dif = im - mc, raw row-grads Di (im), D (dif) ----
                df_g = pmid.tile([P, g, 256], DT, tag="df")
                di_g = pmid.tile([P, g, 256], DT, tag="di")
                dd_g = pmid.tile([P, g, 256], DT, tag="dd")
                nc.vector.tensor_sub(df_g, im_g, mc_g)
                im_rc = im_g.rearrange("p g (r c) -> p g r c", c=16)
                di_rc = di_g.rearrange("p g (r c) -> p g r c", c=16)
                df_rc = df_g.rearrange("p g (r c) -> p g r c", c=16)
                dd_rc = dd_g.rearrange("p g (r c) -> p g r c", c=16)
                nc.vector.tensor_sub(
                    di_g[:, :, 16:240], im_g[:, :, 32:256], im_g[:, :, 0:224]
                )
                nc.vector.tensor_sub(
                    di_rc[:, :, 0:16:15, :],
                    im_rc[:, :, 1:16:14, :],
                    im_rc[:, :, 0:15:14, :],
                )
                nc.vector.tensor_sub(
                    dd_g[:, :, 16:240], df_g[:, :, 32:256], df_g[:, :, 0:224]
                )
                nc.vector.tensor_sub(
                    dd_rc[:, :, 0:16:15, :],
                    df_rc[:, :, 1:16:14, :],
                    df_rc[:, :, 0:15:14, :],
                )
                # D^2 in place with edge-loss weights baked in:
                # interior 0.0625*D^2, edge rows 0.5*D^2
                nc.scalar.activation(
                    out=dd_g[:, :, 16:240],
                    in_=dd_g[:, :, 16:240],
                    func=F.Square,
                    scale=0.25,
                )
                nc.scalar.activation(
                    out=dd_rc[:, :, 0:16:15, :],
                    in_=dd_rc[:, :, 0:16:15, :],
                    func=F.Square,
                    scale=float(np.sqrt(0.5)),
                )

                # ---- per-chunk fused product+sum (STT with accum) ----
                gl1 = glob1[:, g0 : g0 + g, :]
                if os.environ.get("KBATCH_SUMS", "1") == "1":
                    # group-batched: one TT product + one reduce per sum type
                    im4 = im_g.rearrange("p g (r c) -> p g r c", c=16)
                    mc4r = mc_g.rearrange("p g (r c) -> p g r c", c=16)
                    i24 = i2_g.rearrange("p g (r c) -> p g r c", c=16)
                    m44 = m4_g.rearrange("p g (r c) -> p g r c", c=16)
                    m34 = m3_g.rearrange("p g (r c) -> p g r c", c=16)
                    prod = pscr.tile([P, g, 256], DT, tag="prodg")
                    p4d = prod.rearrange("p g (r c) -> p g r c", c=16)

                    def bsum(col, a, b, rows=True):
                        if rows:
                            nc.vector.tensor_mul(prod[:, :, 0:224], a, b)
                            nc.vector.reduce_sum(
                                out=gl1[:, :, col], in_=prod[:, :, 0:224], axis=AX.X
                            )
                        else:
                            nc.vector.tensor_mul(p4d[:, :, :, 0:14], a, b)
                            nc.vector.reduce_sum(
                                out=gl1[:, :, col],
                                in_=p4d[:, :, :, 0:14],
                                axis=AX.XY,
                            )

                    # P_t, A_t, B_t (up,down = row shifts; left,right = col shifts)
                    bsum(0, im_g[:, :, 0:224], mc_g[:, :, 32:256])
                    bsum(1, im_g[:, :, 32:256], mc_g[:, :, 0:224])
                    bsum(2, im4[:, :, :, 0:14], mc4r[:, :, :, 2:16], rows=False)
                    bsum(3, im4[:, :, :, 2:16], mc4r[:, :, :, 0:14], rows=False)
                    bsum(4, i2_g[:, :, 0:224], m4_g[:, :, 32:256])
                    bsum(5, i2_g[:, :, 32:256], m4_g[:, :, 0:224])
                    bsum(6, i24[:, :, :, 0:14], m44[:, :, :, 2:16], rows=False)
                    bsum(7, i24[:, :, :, 2:16], m44[:, :, :, 0:14], rows=False)
                    bsum(8, im_g[:, :, 0:224], m3_g[:, :, 32:256])
                    bsum(9, im_g[:, :, 32:256], m3_g[:, :, 0:224])
                    bsum(10, im4[:, :, :, 0:14], m34[:, :, :, 2:16], rows=False)
                    bsum(11, im4[:, :, :, 2:16], m34[:, :, :, 0:14], rows=False)
                    # p0, pd, edge over full 256
                    nc.vector.tensor_mul(prod, mc_g, im_g)
                    nc.vector.reduce_sum(out=gl1[:, :, 12], in_=prod, axis=AX.X)
                    nc.vector.tensor_mul(
                        prod, mc_g, diam.unsqueeze(1).to_broadcast([P, g, 256])
                    )
                    nc.vector.reduce_sum(out=gl1[:, :, 13], in_=prod, axis=AX.X)
                    nc.vector.tensor_mul(prod, dd_g, di_g)
                    nc.vector.reduce_sum(out=gl1[:, :, 14], in_=prod, axis=AX.X)
                    # consistency relu (batched) + per-group reduce
                    scr_s = pscr.tile([P, g, 128], DT, tag="scrs")
                    nc.scalar.activation(out=scr_s, in_=z_g, func=F.Relu)
                    nc.vector.reduce_sum(out=gl1[:, :, 15], in_=scr_s, axis=AX.X)
                else:
                    self_chunk_sums(nc, pscr, g, im_g, mc_g, i2_g, m4_g, m3_g,
                                    dd_g, di_g, z_g, diam, gl1)
                rs0 = ptiny.tile([P, g], DT, tag="rs0")
                ratio = ptiny.tile([P, g], DT, tag="ratio")
                nc.vector.reciprocal(out=rs0, in_=gl2[:, :, 8])
                nc.vector.tensor_mul(ratio, gl1[:, :, 12], rs0)
                for cg in range(g):
                    nc.scalar.activation(
                        out=img_g[:, cg],
                        in_=mc_g[:, cg],
                        func=F.Copy,
                        scale=ratio[:, cg : cg + 1],
                    )

                nc.sync.dma_start(out=omc_r[:, g0 : g0 + g, :], in_=mc_g)
                nc.sync.dma_start(out=oim_r[:, g0 : g0 + g, :], in_=img_g)

            finish_scalars(nc, glob1, glob2, stag, pglob, nch, osc_r)

    nc.compile()
    return nc


def self_chunk_sums(nc, pscr, g, im_g, mc_g, i2_g, m4_g, m3_g, dd_g, di_g, z_g,
                    diam, gl1):
    for cg in range(g):
        scr = pscr.tile([P, 256], DT, tag="scrv")
        scr_rc = scr.rearrange("p (r c) -> p r c", c=16)
        imC = im_g[:, cg]
        mcC = mc_g[:, cg]
        i2C = i2_g[:, cg]
        m4C = m4_g[:, cg]
        m3C = m3_g[:, cg]
        imR = imC.rearrange("p (r c) -> p r c", c=16)
        mcR = mcC.rearrange("p (r c) -> p r c", c=16)
        i2R = i2C.rearrange("p (r c) -> p r c", c=16)
        m4R = m4C.rearrange("p (r c) -> p r c", c=16)
        m3R = m3C.rearrange("p (r c) -> p r c", c=16)

        def psum(out_ap, a, b, acc):
            nc.vector.scalar_tensor_tensor(
                out=out_ap,
                in0=a,
                scalar=1.0,
                in1=b,
                op0=ALU.mult,
                op1=ALU.mult,
                accum_out=acc,
            )

        # P_t = sum im_sh * mc_sh  (up, down, left, right)
        psum(scr[:, 0:224], imC[:, 0:224], mcC[:, 32:256], gl1[:, cg, 0:1])
        psum(scr[:, 0:224], imC[:, 32:256], mcC[:, 0:224], gl1[:, cg, 1:2])
        psum(scr_rc[:, :, 0:14], imR[:, :, 0:14], mcR[:, :, 2:16], gl1[:, cg, 2:3])
        psum(scr_rc[:, :, 0:14], imR[:, :, 2:16], mcR[:, :, 0:14], gl1[:, cg, 3:4])
        # A_t = sum im^2 * m^4
        psum(scr[:, 0:224], i2C[:, 0:224], m4C[:, 32:256], gl1[:, cg, 4:5])
        psum(scr[:, 0:224], i2C[:, 32:256], m4C[:, 0:224], gl1[:, cg, 5:6])
        psum(scr_rc[:, :, 0:14], i2R[:, :, 0:14], m4R[:, :, 2:16], gl1[:, cg, 6:7])
        psum(scr_rc[:, :, 0:14], i2R[:, :, 2:16], m4R[:, :, 0:14], gl1[:, cg, 7:8])
        # B_t = sum im * m^3
        psum(scr[:, 0:224], imC[:, 0:224], m3C[:, 32:256], gl1[:, cg, 8:9])
        psum(scr[:, 0:224], imC[:, 32:256], m3C[:, 0:224], gl1[:, cg, 9:10])
        psum(scr_rc[:, :, 0:14], imR[:, :, 0:14], m3R[:, :, 2:16], gl1[:, cg, 10:11])
        psum(scr_rc[:, :, 0:14], imR[:, :, 2:16], m3R[:, :, 0:14], gl1[:, cg, 11:12])
        # p0 = sum mc*im ; pd = sum mc*diamond
        psum(scr, mcC, imC, gl1[:, cg, 12:13])
        psum(scr, mcC, diam, gl1[:, cg, 13:14])
        # edge: weights already baked into dd (= scaled D^2)
        psum(scr, dd_g[:, cg], di_g[:, cg], gl1[:, cg, 14:15])
        # consistency relu + accumulate (ScalarE)
        scr_s = pscr.tile([P, 128], DT, tag="scrs")
        nc.scalar.activation(
            out=scr_s,
            in_=z_g[:, cg],
            func=F.Relu,
            accum_out=gl1[:, cg, 15:16],
        )


def finish_scalars(nc, glob1, glob2, stag, pglob, nch, osc_r):
            # ================= final scalar math over all chunks =================
            drv = pglob.tile([P, nch, 16], DT, tag="drv")
            # s_t = S0 - partial ; C_t = M0 - partial(m2)   (t: up,down,left,right)
            for t in range(4):
                nc.vector.tensor_sub(
                    drv[:, :, t], glob2[:, :, 8], glob2[:, :, t]
                )  # s_t
                nc.vector.tensor_sub(
                    drv[:, :, 4 + t], glob2[:, :, 9], glob2[:, :, 4 + t]
                )  # C_t
            # rs_t = 1/s_t
            nc.vector.reciprocal(out=drv[:, :, 8:12], in_=drv[:, :, 0:4])
            # mu_t = P_t * rs_t
            nc.vector.tensor_mul(drv[:, :, 12:16], glob1[:, :, 0:4], drv[:, :, 8:12])

            drv2 = pglob.tile([P, nch, 16], DT, tag="drv2")
            mu = drv[:, :, 12:16]
            # t1 = mu*B ; t2 = A - 2*t1 ; musq = mu*mu ; t4 = musq*C ; t5 = t2+t4
            nc.vector.tensor_mul(drv2[:, :, 0:4], mu, glob1[:, :, 8:12])
            nc.vector.tensor_add(drv2[:, :, 0:4], drv2[:, :, 0:4], drv2[:, :, 0:4])
            nc.vector.tensor_sub(drv2[:, :, 4:8], glob1[:, :, 4:8], drv2[:, :, 0:4])
            nc.vector.tensor_mul(drv2[:, :, 8:12], mu, mu)  # musq
            nc.vector.tensor_mul(drv2[:, :, 12:16], drv2[:, :, 8:12], drv[:, :, 4:8])
            nc.vector.tensor_add(drv2[:, :, 4:8], drv2[:, :, 4:8], drv2[:, :, 12:16])
            # var_t = t5 * rs_t
            nc.vector.tensor_mul(drv2[:, :, 0:4], drv2[:, :, 4:8], drv[:, :, 8:12])

            fin = pglob.tile([P, nch, 8], DT, tag="fin")
            # sums over t of mu, mu^2, v, v^2
            nc.vector.reduce_sum(out=fin[:, :, 0], in_=mu, axis=AX.X)
            nc.vector.reduce_sum(out=fin[:, :, 1], in_=drv2[:, :, 8:12], axis=AX.X)
            nc.vector.reduce_sum(out=fin[:, :, 2], in_=drv2[:, :, 0:4], axis=AX.X)
            nc.vector.tensor_mul(drv2[:, :, 12:16], drv2[:, :, 0:4], drv2[:, :, 0:4])
            nc.vector.reduce_sum(out=fin[:, :, 3], in_=drv2[:, :, 12:16], axis=AX.X)
            # fvar = (smu2+sv2)/8 - (smu^2 + sv^2)/32
            nc.vector.tensor_add(fin[:, :, 4], fin[:, :, 1], fin[:, :, 3])
            nc.vector.tensor_mul(fin[:, :, 5], fin[:, :, 0], fin[:, :, 0])
            nc.vector.tensor_mul(fin[:, :, 6], fin[:, :, 2], fin[:, :, 2])
            # fvar = (smu2+sv2)/8 - (smu^2+sv^2)/32 = (4*tA - tD)/32
            nc.vector.tensor_add(fin[:, :, 5], fin[:, :, 5], fin[:, :, 6])  # tD
            nc.vector.tensor_add(fin[:, :, 4], fin[:, :, 4], fin[:, :, 4])  # 2*tA
            nc.vector.tensor_add(fin[:, :, 4], fin[:, :, 4], fin[:, :, 4])  # 4*tA
            nc.vector.tensor_sub(fin[:, :, 4], fin[:, :, 4], fin[:, :, 5])
            nc.scalar.activation(
                out=stag[:, :, 2], in_=fin[:, :, 4], func=F.Copy, scale=1.0 / 32.0
            )
            # consistency = relu_sum / 128
            nc.scalar.activation(
                out=stag[:, :, 0], in_=glob1[:, :, 15], func=F.Copy, scale=1.0 / 128.0
            )
            # rounding = (S0 - M0)/64 - 1
            nc.vector.tensor_sub(fin[:, :, 7], glob2[:, :, 8], glob2[:, :, 9])
            nc.scalar.activation(
                out=stag[:, :, 1],
                in_=fin[:, :, 7],
                func=F.Copy,
                scale=1.0 / 64.0,
                bias=-1.0,
            )
            # edge = e2 / 256
            nc.scalar.activation(
                out=stag[:, :, 3], in_=glob1[:, :, 14], func=F.Copy, scale=1.0 / 256.0
            )
            # avg_cov = (M0 - 2*pd + K) * 0.5/256
            nc.vector.tensor_add(fin[:, :, 6], glob1[:, :, 13], glob1[:, :, 13])
            nc.vector.tensor_sub(fin[:, :, 6], glob2[:, :, 9], fin[:, :, 6])
            nc.scalar.activation(
                out=stag[:, :, 4],
                in_=fin[:, :, 6],
                func=F.Copy,
                scale=0.5 / 256.0,
                bias=K_D2 * 0.5 / 256.0,
            )

            nc.sync.dma_start(out=osc_r, in_=stag)


@functools.lru_cache(maxsize=2)
def _get_nc(npc: int, g: int):
    return build_nc(npc, g)


def _pick_g(npc: int) -> int:
    for g in (8, 4, 2, 1):
        if npc % (P * g) == 0:
            return g
    raise ValueError(f"npc={npc} not divisible by {P}")


def run_cores(resized_image, mask_new, mask_old, trace=False, ncores=NCORES):
    B = resized_image.shape[0]
    assert B % ncores == 0
    npc = B // ncores
    nc = _get_nc(npc, _pick_g(npc))
    im = np.ascontiguousarray(resized_image.reshape(B, 256), dtype=np.float32)
    mn = np.ascontiguousarray(mask_new.reshape(B, 128), dtype=np.float32)
    mo = np.ascontiguousarray(mask_old.reshape(B, 128), dtype=np.float32)
    dm = np.ascontiguousarray(np.tile(DIAM.reshape(1, 256), (P, 1)))
    in_maps = [
        {
            "x_im": im[c * npc : (c + 1) * npc],
            "x_mn": mn[c * npc : (c + 1) * npc],
            "x_mo": mo[c * npc : (c + 1) * npc],
            "c_dm": dm,
        }
        for c in range(ncores)
    ]
    res = run_bass_kernel_spmd(
        nc, in_maps, core_ids=list(range(ncores)), trace=trace
    )
    outs = res.results
    mc = np.concatenate([r["o_mc"] for r in outs]).reshape(B, 16, 16)
    oi = np.concatenate([r["o_im"] for r in outs]).reshape(B, 16, 16)
    sc = np.concatenate([r["o_sc"] for r in outs])
    result = (
        mc,
        oi,
        np.ascontiguousarray(sc[:, 0]),
        np.ascontiguousarray(sc[:, 1]),
        np.ascontiguousarray(sc[:, 2]),
        np.ascontiguousarray(sc[:, 3]),
        np.ascontiguousarray(sc[:, 4]),
    )
    return result, res


def kernel(resized_image, mask_new, mask_old, **_unused):
    result, _ = run_cores(resized_image, mask_new, mask_old)
    return result
